# revision 33
# baseline (speedup 1.0000x reference)
"""Trainium2 Bass kernel for nn_FLinemodel_37185826849029.

Model (per batch b):
  Q = x@wq, K = x@wk, V = x@wv          [S,4]
  L = (Q K^T) @ W_at + b_at             [S,S]   <- rewritten as Q @ (K^T W_at)
  A = softmax(L, axis=-1)
  y = A @ V                             [S,4]
  p = softmax(y @ w_cls + b_cls)        [S,10]
  out = mean_s p                        [10]

Key algebraic rewrite: (Q K^T) W_at == Q (K^T W_at) since D=4, collapsing
~550 GFLOP to ~4 GFLOP.  Everything runs in a "transposed" layout (L^T
tiles [u,s]) so the softmax denominator comes free from the attend matmul
via a ones-column appended to V, and the classifier-stage sums come from a
unit column appended to w_cls.

Sequence axes use interleaved grouping: partition g holds rows {128*r + g}
(r = 0..R-1), so u-tiles are contiguous 128-blocks (tile rt = u in
[128*rt, 128*rt+128)) and W_at can be streamed in contiguous column slabs.
The s axis (queries) is permutation-invariant through the final mean.

Performance structure:
  - x / W_at / wq / wk / wv are converted to bf16 on the host: halves HBM
    traffic and runs all large matmuls at 1 cycle/row on the PE.
  - DMAs spread over three queues: x on the ACT queue, W_at u-slabs on the
    Pool queue, small weights on SP - all in flight from t=0.
  - Stage 2 (M = K^T W_at) is chunked by u-slab and interleaved into the
    first stage-3 pass, so the PE absorbs it in ACT slack.
  - Stage 3 is ACT-bound (16.8M exps): sc-outer / half-batch exp groups with
    double-buffered L^T PSUM tiles keep the scalar engine streaming ~100%.
  - The epilogue is split into per-sc chunks interleaved with stage 3.

Sharding: data-parallel over batch. 32 batches / 8 cores = 4 per core;
batches are packed into the PE array concurrently via tile_position.
"""

from contextlib import ExitStack

import numpy as np

import concourse.bacc as bacc
import concourse.mybir as mybir
import concourse.tile as tile
from concourse import masks

F32 = mybir.dt.float32
BF16 = mybir.dt.bfloat16
EXP = mybir.ActivationFunctionType.Exp
P = 128


B, S_FULL, H_FULL, D, C = 32, 2048, 256, 4, 10
N_CORES = 8
B_LOC = B // N_CORES


def build_nc(b_loc=B_LOC, s=S_FULL, h=H_FULL, reps=1):
    HC = h // P            # h chunks (2)
    R = s // P             # rows per partition; also # of u-tiles
    SC = s // 512          # 512-wide s chunks
    E = C + 1
    assert HC == 2 and s % 512 == 0 and R % 2 == 0

    nc = bacc.Bacc("TRN2", debug=False, target_bir_lowering=False)

    xs_t = nc.dram_tensor("xs", [b_loc, h, s], BF16, kind="ExternalInput")
    wq_t = nc.dram_tensor("wq", [h, D], BF16, kind="ExternalInput")
    wk_t = nc.dram_tensor("wk", [h, D], BF16, kind="ExternalInput")
    wv_t = nc.dram_tensor("wv", [h, D], BF16, kind="ExternalInput")
    wat_t = nc.dram_tensor("w_at", [s, s], BF16, kind="ExternalInput")
    bat_t = nc.dram_tensor("b_at", [s], F32, kind="ExternalInput")
    wcls_t = nc.dram_tensor("w_cls", [D, C], F32, kind="ExternalInput")
    bcls_t = nc.dram_tensor("b_cls", [C], F32, kind="ExternalInput")
    out_t = nc.dram_tensor("out", [b_loc, C], F32, kind="ExternalOutput")

    xs, wat = xs_t.ap(), wat_t.ap()

    with ExitStack() as ctx:
        tc = ctx.enter_context(tile.TileContext(nc))
        const = ctx.enter_context(tc.tile_pool(name="const", bufs=1))
        big = ctx.enter_context(tc.tile_pool(name="big", bufs=1))

        ones_col = const.tile([P, 1], F32)
        nc.vector.memset(ones_col[:], 1.0)

        # w{q,kv}_sb[p, hc, d] = w[hc*P + p, d]; direct strided bf16 loads.
        wq_sb = const.tile([P, HC, D], BF16)
        wkv_sb = const.tile([P, HC, 2 * D], BF16)
        nc.gpsimd.dma_start(
            wq_sb[:], wq_t.ap().rearrange("(hh p) d -> p hh d", p=P))
        nc.gpsimd.dma_start(
            wkv_sb[:, :, 0:D], wk_t.ap().rearrange("(hh p) d -> p hh d", p=P))
        nc.gpsimd.dma_start(
            wkv_sb[:, :, D : 2 * D], wv_t.ap().rearrange("(hh p) d -> p hh d", p=P))

        # bat_sb[g, rt] = b_at[128*rt + g]  (contiguous u order); loaded in
        # the rep-0 Pool stream after the first W_at slab
        bat_sb = const.tile([P, R], F32)

        # classifier weights extended: rows (d0..d3, bias), cols (c0..c9, unit),
        # replicated at partition strips 32b for the row-tiled z matmuls.
        wce = const.tile([P, b_loc * E], F32)

        qt_sb = big.tile([P, s], BF16)              # rows 32b..: Q_b^T [4, s']
        k_pad = big.tile([P, R, P], BF16)           # col 32b+d = K_b[128r+g, d], else 0
        v_sb = big.tile([P, b_loc, R, D + 1], BF16) # V[128rt+g, d] + ones col
        m_sb = big.tile([P, s], BF16)               # rows 32b..: M^T, col u (contig)
        yt_sb = big.tile([P, s], F32)               # rows 32b..: [yhat^T; rowsum]
        out_acc = big.tile([1, b_loc * C], F32)     # mean accumulator (SBUF)
        wg = big.tile([P, R, s], BF16)              # wg[g, r, u] = W_at[128r+g, u]

        nc.vector.memset(v_sb[:], 1.0)
        nc.vector.memset(k_pad[:], 0.0)

        wat_g = wat.rearrange("(r g) u -> g r u", g=P)

        for _rep in range(reps):
            # W_at u-slab prefetches on the Pool DMA queue (in flight from
            # t=0); slab 0 split in r-halves so stage-2 uc0 terms can start
            # as soon as stage 1 produces K rows
            nc.gpsimd.dma_start(wg[:, : R // 2, 0:512], wat_g[:, : R // 2, 0:512])
            nc.gpsimd.dma_start(wg[:, R // 2 :, 0:512], wat_g[:, R // 2 :, 0:512])
            if _rep == 0:
                nc.gpsimd.dma_start(
                    bat_sb[:], bat_t.ap().rearrange("(r g) -> g r", g=P))
            for uc in range(1, SC):
                nc.gpsimd.dma_start(
                    wg[:, :, 512 * uc : 512 * (uc + 1)],
                    wat_g[:, :, 512 * uc : 512 * (uc + 1)],
                )
            if _rep == 0:
                # classifier weights, queued on Pool behind the W_at slabs
                # (needed only by the epilogue at ~60us)
                nc.vector.memset(wce[:], 0.0)
                for b in range(b_loc):
                    nc.gpsimd.dma_start(
                        wce[32 * b : 32 * b + D, E * b : E * b + C], wcls_t.ap())
                    nc.gpsimd.dma_start(
                        wce[32 * b + D : 32 * b + D + 1, E * b : E * b + C],
                        bcls_t.ap()[None, :])
                    # unit entry at (row D, col E*b + C) of strip b
                    nc.gpsimd.affine_select(
                        out=wce[32 * b : 32 * b + 32, :],
                        in_=wce[32 * b : 32 * b + 32, :],
                        pattern=[[1, b_loc * E]],
                        compare_op=mybir.AluOpType.not_equal,
                        fill=1.0,
                        base=-(b_loc * E * D + E * b + C),
                        channel_multiplier=b_loc * E,
                    )

            # ---- stage 1: x^T chunk loads (pre-transposed on host), K/V/Q;
            # stage-2 uc0 accumulated incrementally, off the critical path ----
            with tc.tile_pool(name="ps_qt", bufs=2, space="PSUM") as ps_qt, \
                 tc.tile_pool(name="ps_m0", bufs=1, space="PSUM") as ps_m0, \
                 tc.tile_pool(name="ps_kv", bufs=1, space="PSUM") as ps_kv:
                kv_full = ps_kv.tile([P, 512], F32)
                kv_ps = kv_full[:, : R * b_loc * 2 * D].rearrange(
                    "p (r b e) -> p r b e", r=R, b=b_loc)
                m0_ps = ps_m0.tile([P, 512], F32)
                RC = R // 4
                k_pad_v = k_pad[:].rearrange("p r (b e) -> p r b e", e=32)
                # x^T loads in half-batch slabs: the first-half slabs of all
                # batches land by ~4us (unblocking rc 0/1) while keeping DMA
                # setup overhead low
                xT_full = []
                for b in range(b_loc):
                    xT_b = big.tile([P, HC, s], BF16, tag=f"xTb{b}", name="xTb")
                    xT_full.append(xT_b)
                xsv = [xs[b].rearrange("(hh p) u -> p hh u", p=P)
                       for b in range(b_loc)]
                for sh in range(2):
                    for b in range(b_loc):
                        xq = nc.scalar if b % 2 == 0 else nc.sync
                        xq.dma_start(
                            xT_full[b][:, :, 1024 * sh : 1024 * (sh + 1)],
                            xsv[b][:, :, 1024 * sh : 1024 * (sh + 1)],
                        )
                xT_tiles = {
                    (rc, b): xT_full[b][:, :, 512 * rc : 512 * (rc + 1)]
                    for rc in range(RC) for b in range(b_loc)
                }
                for rc in range(RC):
                    for b in range(b_loc):
                        xT_c = xT_tiles[(rc, b)]
                        for rloc in range(4):
                            r = 4 * rc + rloc
                            for hc in range(HC):
                                nc.tensor.matmul(
                                    kv_ps[:, r, b, :],
                                    xT_c[:, hc, rloc * P : (rloc + 1) * P],
                                    wkv_sb[:, hc, :],
                                    start=(hc == 0),
                                    stop=(hc == HC - 1),
                                )
                    # qt slabs 0,1 here (needed by stage-3 passes 0,1); slabs
                    # 2,3 are injected into stage 3 to shorten the prologue
                    if rc < 2:
                        qt_t = ps_qt.tile([P, 512], F32, tag="qt", name="qt")
                        nc.vector.memset(qt_t[:], 0.0)
                        for b in range(b_loc):
                            for hc in range(HC):
                                nc.tensor.matmul(
                                    qt_t[32 * b : 32 * b + D, :],
                                    wq_sb[:, hc, :],
                                    xT_tiles[(rc, b)][:, hc, :],
                                    start=(hc == 0),
                                    stop=(hc == HC - 1),
                                    skip_group_check=True,
                                    tile_position=(0, 32 * b),
                                )
                        nc.vector.tensor_copy(
                            qt_sb[:, 512 * rc : 512 * (rc + 1)], qt_t[:])

                    nc.vector.tensor_copy(
                        k_pad_v[:, 4 * rc : 4 * rc + 4, 0:b_loc, 0:D],
                        kv_ps[:, 4 * rc : 4 * rc + 4, :, 0:D],
                    )
                    nc.vector.tensor_copy(
                        v_sb[:, :, 4 * rc : 4 * rc + 4, 0:D],
                        kv_ps[:, 4 * rc : 4 * rc + 4, :, D : 2 * D].rearrange(
                            "p r b d -> p b r d"),
                    )
                    # uc0 terms for the PREVIOUS rc: overlaps this rc's k_pad
                    # copy with PE work instead of stalling on it
                    for rloc in range(4):
                        r = 4 * (rc - 1) + rloc
                        if 0 <= r:
                            nc.tensor.matmul(
                                m0_ps[:],
                                k_pad[:, r, :],
                                wg[:, r, 0:512],
                                start=(r == 0),
                                stop=False,
                            )
                for rloc in range(4):
                    r = 4 * (RC - 1) + rloc
                    nc.tensor.matmul(
                        m0_ps[:],
                        k_pad[:, r, :],
                        wg[:, r, 0:512],
                        start=False,
                        stop=(r == R - 1),
                    )
                nc.vector.tensor_copy(m_sb[:, 0:512], m0_ps[:])

            # ---- stages 2+3 fused: M chunks + attend + epilogue, interleaved --
            # Stage 3 steps (sc, rt, h): 2 L^T matmuls (PE) -> exp (ACT) -> 2 y
            # accumulations (PE).  ACT is the bottleneck (~1.05us/step); the PE
            # has ~0.2us/step slack which absorbs the interleaved stage-2 u-slab
            # matmuls and epilogue chunks.
            HB = 2
            NH = b_loc // HB
            SPC = R * NH  # steps per sc pass
            with tc.tile_pool(name="esb", bufs=3) as e_pool, \
                 tc.tile_pool(name="ps_y", bufs=1, space="PSUM") as ps_y, \
                 tc.tile_pool(name="ps_l", bufs=2, space="PSUM") as ps_l, \
                 tc.tile_pool(name="ps_z", bufs=1, space="PSUM") as ps_z, \
                 tc.tile_pool(name="ep", bufs=2) as ep:
                steps = [(sc, rt, h) for sc in range(SC) for rt in range(R)
                         for h in range(NH)]
                y_tiles, l_tiles, e_tiles = {}, {}, {}

                def emit_l(n):
                    sc, rt, h = steps[n]
                    if (rt, h) == (0, 0):
                        y_tiles[sc] = ps_y.tile([P, 512], F32, tag="y", name="y")
                        nc.vector.memset(y_tiles[sc][:], 0.0)
                    l_ps = ps_l.tile([P, HB, 512], F32, tag="l", name="l")
                    l_tiles[n] = l_ps
                    for j in range(HB):
                        i = h * HB + j
                        nc.tensor.matmul(
                            l_ps[:, j, :],
                            m_sb[32 * i : 32 * i + D, rt * P : (rt + 1) * P],
                            qt_sb[32 * i : 32 * i + D, 512 * sc : 512 * (sc + 1)],
                            start=True,
                            stop=True,
                            tile_position=(32 * i, 0),
                        )

                def emit_exp(n):
                    sc, rt, h = steps[n]
                    e_sb = e_pool.tile([P, HB, 512], BF16, tag="e", name="e")
                    e_tiles[n] = e_sb
                    nc.scalar.activation(
                        e_sb[:], l_tiles.pop(n)[:], EXP,
                        bias=bat_sb[:, rt : rt + 1], scale=1.0,
                    )

                def emit_y(n):
                    sc, rt, h = steps[n]
                    for j in range(HB):
                        i = h * HB + j
                        nc.tensor.matmul(
                            y_tiles[sc][32 * i : 32 * i + D + 1, :],
                            v_sb[:, i, rt, :],
                            e_tiles.pop(n)[:, j, :] if j == HB - 1 else
                            e_tiles[n][:, j, :],
                            start=(rt == 0),
                            stop=(rt == R - 1),
                            skip_group_check=True,
                            tile_position=(0, 32 * i),
                        )
                    if (rt, h) == (R - 1, NH - 1):
                        nc.vector.tensor_copy(
                            yt_sb[:, 512 * sc : 512 * (sc + 1)], y_tiles[sc][:]
                        )

                # stage-2 u-slab: m[:, slab] = sum_r k_pad[:,r]^T @ wg[:,r,slab].
                # Split into r-halves so injected PE work stays under the
                # pipeline's ACT-slack absorption depth.  m tiles live in the
                # ps_l pool under their own tag (2 extra banks).
                m_holder = {}

                def emit_m(uc, q):
                    if q == 0:
                        m_holder[uc] = ps_l.tile(
                            [P, 512], F32, tag="m", name="m", bufs=2)
                    m_ps = m_holder[uc]
                    for r in range(4 * q, 4 * q + 4):
                        nc.tensor.matmul(
                            m_ps[:],
                            k_pad[:, r, :],
                            wg[:, r, 512 * uc : 512 * (uc + 1)],
                            start=(r == 0),
                            stop=(r == R - 1),
                        )
                    if q == 3:
                        nc.vector.tensor_copy(
                            m_sb[:, 512 * uc : 512 * (uc + 1)], m_holder.pop(uc)[:])

                # qt slab injection (slabs 2,3): 2 half-batch groups + copy
                qt_holder = {}

                def emit_q(rc, half):
                    if half == 0:
                        qt_holder[rc] = ps_l.tile(
                            [P, 512], F32, tag="m", name="qtl", bufs=2)
                        nc.vector.memset(qt_holder[rc][:], 0.0)
                    qt_t = qt_holder[rc]
                    for b in (0, 1) if half == 0 else (2, 3):
                        for hc in range(HC):
                            nc.tensor.matmul(
                                qt_t[32 * b : 32 * b + D, :],
                                wq_sb[:, hc, :],
                                xT_tiles[(rc, b)][:, hc, :],
                                start=(hc == 0),
                                stop=(hc == HC - 1),
                                skip_group_check=True,
                                tile_position=(0, 32 * b),
                            )
                    if half == 1:
                        nc.vector.tensor_copy(
                            qt_sb[:, 512 * rc : 512 * (rc + 1)],
                            qt_holder.pop(rc)[:])

                # epilogue chunk for one sc pass: z, softmax, partial mean
                KR = R // SC

                def emit_ep(kh, ps_z):
                    z_full = ps_z.tile([P, 512], F32, tag="zf", name="zf")
                    z_ps = z_full[:, : KR * b_loc * E].rearrange(
                        "p (k i e) -> p k i e", k=KR, i=b_loc)
                    for kk in range(KR):
                        k = kh * KR + kk
                        nc.tensor.matmul(
                            z_ps[:, kk, :, :].rearrange("p i e -> p (i e)"),
                            yt_sb[:, k * P : (k + 1) * P],
                            wce[:],
                            start=True,
                            stop=True,
                        )
                    r_sb = ep.tile([P, KR * b_loc], F32, tag="r", name="r")
                    nc.vector.reciprocal(r_sb[:], z_ps[:, :, :, C])
                    zz = ep.tile([P, KR, b_loc, C], F32, tag="zz", name="zz")
                    nc.vector.tensor_tensor(
                        zz[:],
                        z_ps[:, :, :, 0:C],
                        r_sb[:].rearrange("p (k i) -> p k i", k=KR)
                            .unsqueeze(-1).broadcast_to([P, KR, b_loc, C]),
                        mybir.AluOpType.mult,
                    )
                    ez = ep.tile([P, KR, b_loc, C], F32, tag="ez", name="ez")
                    nc.scalar.activation(ez[:], zz[:], EXP)
                    sz = ep.tile([P, KR * b_loc], F32, tag="sz", name="sz")
                    nc.vector.tensor_reduce(
                        sz[:], ez[:], axis=mybir.AxisListType.X, op=mybir.AluOpType.add
                    )
                    rz = ep.tile([P, KR * b_loc], F32, tag="rz", name="rz")
                    nc.vector.reciprocal(rz[:], sz[:])
                    pz = ep.tile([P, KR, b_loc, C], F32, tag="pz", name="pz")
                    nc.vector.tensor_tensor(
                        pz[:],
                        ez[:],
                        rz[:].rearrange("p (k i) -> p k i", k=KR)
                            .unsqueeze(-1).broadcast_to([P, KR, b_loc, C]),
                        mybir.AluOpType.mult,
                    )
                    pc_sb = ep.tile([P, b_loc, C], F32, tag="pc", name="pc")
                    nc.vector.tensor_reduce(
                        pc_sb[:],
                        pz[:].rearrange("p k i c -> p i c k"),
                        axis=mybir.AxisListType.X,
                        op=mybir.AluOpType.add,
                    )
                    # partial mean into free z-tile columns, accumulated in SBUF
                    mslot = z_full[0:1, 448 : 448 + b_loc * C]
                    nc.tensor.matmul(
                        mslot,
                        ones_col[:],
                        pc_sb[:].rearrange("p i c -> p (i c)"),
                        start=True,
                        stop=True,
                    )
                    if kh == 0:
                        nc.vector.tensor_copy(out_acc[:], mslot)
                    else:
                        nc.vector.tensor_tensor(
                            out_acc[:], out_acc[:], mslot, mybir.AluOpType.add)
                    if kh == SC - 1:
                        out_sb = ep.tile([1, b_loc * C], F32, tag="o", name="o")
                        nc.scalar.mul(out_sb[:], out_acc[:], 1.0 / s)
                        nc.sync.dma_start(
                            out_t.ap().rearrange("b c -> (b c)")[None, :], out_sb[:])

                emit_l(0)
                emit_l(1)
                for n in range(len(steps)):
                    emit_exp(n)
                    if n + 2 < len(steps):
                        emit_l(n + 2)
                    emit_y(n)
                    # inject remaining stage-2 quarter-slabs across the first
                    # sc pass (uc slab needed by the l() emitted at n = 8*uc-2)
                    m_inj = {0: (1, 0), 1: (1, 1), 3: (1, 2), 5: (1, 3),
                             7: (2, 0), 9: (2, 1), 11: (2, 2), 13: (2, 3),
                             15: (3, 0), 17: (3, 1), 19: (3, 2), 21: (3, 3)}
                    if n in m_inj:
                        emit_m(*m_inj[n])
                    # inject qt slabs 2,3 during passes 1,2
                    q_inj = {40: (2, 0), 43: (2, 1), 72: (3, 0), 75: (3, 1)}
                    if n in q_inj:
                        emit_q(*q_inj[n])
                    # epilogue chunk after each completed sc pass
                    if (n + 1) % SPC == 0:
                        emit_ep((n + 1) // SPC - 1, ps_z)

    nc.finalize()
    return nc


_NC_CACHE = {}


def _get_nc(key=(B_LOC, S_FULL, H_FULL), reps=1):
    if (key, reps) not in _NC_CACHE:
        _NC_CACHE[(key, reps)] = build_nc(*key, reps=reps)
    return _NC_CACHE[(key, reps)]


def _bf16(a):
    import ml_dtypes

    return np.ascontiguousarray(np.asarray(a, np.float32)).astype(ml_dtypes.bfloat16)


def kernel(x, wq, wk, wv, w_at, b_at, w_cls, b_cls):
    from concourse.bass_utils import run_bass_kernel_spmd

    # ship x transposed to [B, H, S]: stage 1 loads x^T slabs directly,
    # skipping all on-chip transposes
    x = _bf16(np.asarray(x, np.float32).transpose(0, 2, 1))
    nc = _get_nc()
    shared = {
        "wq": _bf16(wq),
        "wk": _bf16(wk),
        "wv": _bf16(wv),
        "w_at": _bf16(w_at),
        "b_at": np.asarray(b_at, np.float32),
        "w_cls": np.asarray(w_cls, np.float32),
        "b_cls": np.asarray(b_cls, np.float32),
    }
    in_maps = [
        {"xs": x[c * B_LOC : (c + 1) * B_LOC], **shared} for c in range(N_CORES)
    ]
    last_err = None
    for _attempt in range(3):
        try:
            res = run_bass_kernel_spmd(nc, in_maps, list(range(N_CORES))).results
            return np.concatenate([res[c]["out"] for c in range(N_CORES)], axis=0)
        except Exception as e:  # transient NRT/axon execution failures
            last_err = e
    raise last_err
